# revision 3
# baseline (speedup 1.0000x reference)
"""DeepseekV3 MoE MLP (grouped ragged GEMM) on 8 Trainium2 NeuronCores.

Strategy: expert-parallel. 32 experts / 8 cores = 4 experts per core; each
core processes its experts' token groups (tokens arrive pre-sorted by
expert). Compute in bf16 (fp32 accumulation in PSUM), bf16 output DMA
(host upcasts to fp32).

Per-core pipeline, per expert (H=2048, I=1408, C tokens padded):
  stage 1:  gateT[i,t] = sum_h W1[h,i] * XT[h,t]   (W1 tile = lhsT, XT = rhs)
            upT  [i,t] = sum_h W2[h,i] * XT[h,t]
            h2T  [i,t] = silu(gateT) * upT          (ScalarE Silu + VectorE mul)
  stage 2:  down [t,h] = sum_i h2T[i,t] * W3[i,h]   (h2T tile = lhsT, W3 = rhs)

All operands are laid out host-side so every DMA is 128 partitions x
contiguous per-partition blocks; no on-device transposes anywhere.

v2 perf changes vs v1:
 - expert 0's critical-path loads (xt + w1/w2 of the first i-tile) are
   sliced into pieces and issued from 4 different sequencers in parallel
   so the first matmul starts ~10us earlier.
 - w1/w2 are interleaved host-side into one dram tensor -> one DMA per
   i-tile instead of two.
 - w3 is loaded in 2 chunks of 1024 columns instead of 4 of 512.
 - output is written bf16 (half the bytes, half the out DMAs); the host
   upcasts to fp32.  Fewer DMAs also shrink the end-of-program semaphore
   reset train.
"""

import numpy as np
import ml_dtypes

# Problem constants (hardcoded per contract).
E = 32          # experts
H = 2048        # hidden dim
I = 1408        # moe intermediate dim
N_CORES = 8
EPC = E // N_CORES  # experts per core
P = 128
HO = H // P     # 16 h-subtiles
IT = I // P     # 11 i-subtiles
HC2 = H // 1024  # 2 output h-chunks of 1024

BF16 = ml_dtypes.bfloat16

_PROGRAM_CACHE: dict = {}


def _build_program(C: int):
    """Build + compile the per-core Bass program for per-expert capacity C
    (multiple of 512)."""
    import concourse.bacc as bacc
    import concourse.mybir as mybir
    import concourse.tile as tile

    NT = C // 512   # stage-1 token chunks of 512
    TT = C // P     # stage-2 token tiles of 128

    nc = bacc.Bacc("TRN2", debug=False, num_devices=N_CORES)

    xt = nc.dram_tensor("xt", [EPC * NT, P, HO, 512], mybir.dt.bfloat16,
                        kind="ExternalInput").ap()
    w12 = nc.dram_tensor("w12", [EPC, IT, P, 2, HO, P], mybir.dt.bfloat16,
                         kind="ExternalInput").ap()
    w3 = nc.dram_tensor("w3", [EPC, HC2, P, IT, 1024], mybir.dt.bfloat16,
                        kind="ExternalInput").ap()
    out = nc.dram_tensor("out", [EPC * C, H], mybir.dt.bfloat16,
                         kind="ExternalOutput").ap()

    with tile.TileContext(nc) as tc:
        with (
            tc.tile_pool(name="xt0_pool", bufs=4) as xt0_pool,
            tc.tile_pool(name="w10_pool", bufs=4) as w10_pool,
            tc.tile_pool(name="w20_pool", bufs=2) as w20_pool,
            tc.tile_pool(name="xt_pool", bufs=2) as xt_pool,
            tc.tile_pool(name="w12_pool", bufs=3) as w12_pool,
            tc.tile_pool(name="w3_pool", bufs=2) as w3_pool,
            tc.tile_pool(name="h2t_pool", bufs=2) as h2t_pool,
            tc.tile_pool(name="act_pool", bufs=3) as act_pool,
            tc.tile_pool(name="out_pool", bufs=4) as out_pool,
            tc.tile_pool(name="ps_g", bufs=2, space="PSUM") as ps_g,
            tc.tile_pool(name="ps_u", bufs=2, space="PSUM") as ps_u,
            tc.tile_pool(name="ps_d", bufs=3, space="PSUM") as ps_d,
        ):
            # ---- expert 0 fast start: sliced loads from 4 sequencers ----
            # xt chunk 0 in 4 pieces of [128, 4, 512]; w1[e0,it0] in 4
            # pieces of [128, 4, 128]; w2[e0,it0] in 2 pieces of
            # [128, 8, 128].  Each engine's queue issues independently.
            issue_eng = [nc.sync, nc.scalar, nc.gpsimd, nc.sync]
            xt0_tiles = []
            for q in range(4):
                t_sb = xt0_pool.tile([P, 4, 512], mybir.dt.bfloat16, tag="xt0")
                issue_eng[q].dma_start(out=t_sb[:], in_=xt[0][:, 4 * q:4 * q + 4])
                xt0_tiles.append(t_sb)
            w10_tiles = []
            for q in range(4):
                t_sb = w10_pool.tile([P, 4, P], mybir.dt.bfloat16, tag="w10")
                issue_eng[q].dma_start(out=t_sb[:],
                                       in_=w12[0, 0][:, 0, 4 * q:4 * q + 4])
                w10_tiles.append(t_sb)
            w20_tiles = []
            for q in range(2):
                t_sb = w20_pool.tile([P, 8, P], mybir.dt.bfloat16, tag="w20")
                issue_eng[2 * q].dma_start(out=t_sb[:],
                                           in_=w12[0, 0][:, 1, 8 * q:8 * q + 8])
                w20_tiles.append(t_sb)

            # remaining xt chunks of expert 0 (C > 512 only)
            xt0_rest = []
            for tch in range(1, NT):
                t_sb = xt_pool.tile([P, HO, 512], mybir.dt.bfloat16, tag="xt")
                nc.sync.dma_start(out=t_sb[:], in_=xt[tch])
                xt0_rest.append(t_sb)

            def xt_rhs_e0(tch, ho):
                if tch == 0:
                    return xt0_tiles[ho // 4][:, ho % 4]
                return xt0_rest[tch - 1][:, ho]

            for e in range(EPC):
                if e > 0:
                    # ---- load this expert's XT token chunks ----
                    xt_tiles = []
                    for tch in range(NT):
                        t_sb = xt_pool.tile([P, HO, 512], mybir.dt.bfloat16,
                                            tag="xt")
                        nc.sync.dma_start(out=t_sb[:], in_=xt[e * NT + tch])
                        xt_tiles.append(t_sb)

                    def xt_rhs(tch, ho, _tiles=xt_tiles):
                        return _tiles[tch][:, ho]
                else:
                    xt_rhs = xt_rhs_e0

                h2t = h2t_pool.tile([P, IT, C], mybir.dt.bfloat16, tag="h2t")

                # ---- stage 1: gateT/upT + silu*mul -> h2T ----
                for it in range(IT):
                    if e == 0 and it == 0:
                        def w1_lhsT(ho):
                            return w10_tiles[ho // 4][:, ho % 4]

                        def w2_lhsT(ho):
                            return w20_tiles[ho // 8][:, ho % 8]
                    else:
                        w12_sb = w12_pool.tile([P, 2, HO, P],
                                               mybir.dt.bfloat16, tag="w12")
                        nc.sync.dma_start(out=w12_sb[:], in_=w12[e, it])

                        def w1_lhsT(ho, _sb=w12_sb):
                            return _sb[:, 0, ho]

                        def w2_lhsT(ho, _sb=w12_sb):
                            return _sb[:, 1, ho]

                    for tch in range(NT):
                        pg = ps_g.tile([P, 512], mybir.dt.float32, tag="pg")
                        pu = ps_u.tile([P, 512], mybir.dt.float32, tag="pu")
                        for ho in range(HO):
                            nc.tensor.matmul(pg, w1_lhsT(ho), xt_rhs(tch, ho),
                                             start=(ho == 0), stop=(ho == HO - 1))
                        for ho in range(HO):
                            nc.tensor.matmul(pu, w2_lhsT(ho), xt_rhs(tch, ho),
                                             start=(ho == 0), stop=(ho == HO - 1))
                        sil = act_pool.tile([P, 512], mybir.dt.float32, tag="sil")
                        nc.scalar.activation(sil, pg,
                                             mybir.ActivationFunctionType.Silu)
                        nc.vector.tensor_mul(
                            h2t[:, it, tch * 512:(tch + 1) * 512], sil, pu)

                # ---- stage 2: down = h2 @ W3 ----
                for hc in range(HC2):
                    w3_sb = w3_pool.tile([P, IT, 1024], mybir.dt.bfloat16,
                                         tag="w3")
                    nc.sync.dma_start(out=w3_sb[:], in_=w3[e, hc])
                    for tt in range(TT):
                        ot = out_pool.tile([P, 1024], mybir.dt.bfloat16,
                                           tag="ot")
                        for half in range(2):
                            pd = ps_d.tile([P, 512], mybir.dt.float32, tag="pd")
                            for io in range(IT):
                                nc.tensor.matmul(
                                    pd, h2t[:, io, tt * P:(tt + 1) * P],
                                    w3_sb[:, io, half * 512:(half + 1) * 512],
                                    start=(io == 0), stop=(io == IT - 1))
                            nc.scalar.copy(ot[:, half * 512:(half + 1) * 512],
                                           pd)
                        nc.sync.dma_start(
                            out=out[e * C + tt * P: e * C + (tt + 1) * P,
                                    hc * 1024:(hc + 1) * 1024],
                            in_=ot[:])

    nc.compile()
    return nc


def _prep_inputs(hidden_states, gate_w, up_w, down_w, group_sizes, C):
    """Host-side: group tokens by expert (padded to C), transpose, convert to
    bf16, and pre-tile everything into the DMA layouts the program expects."""
    T = hidden_states.shape[0]
    gs = np.asarray(group_sizes, dtype=np.int64)
    offsets = np.zeros(E + 1, dtype=np.int64)
    np.cumsum(gs, out=offsets[1:])
    assert offsets[-1] == T, f"group_sizes sum {offsets[-1]} != T {T}"

    # Pad each expert's token block to C rows, convert to bf16.
    x_pad = np.zeros((E, C, H), dtype=BF16)
    for e in range(E):
        x_pad[e, :gs[e]] = hidden_states[offsets[e]:offsets[e + 1]]

    NT = C // 512
    # XT layout: [core][e_local*NT + tch][p][ho][512] with h = ho*128 + p
    xt_all = np.ascontiguousarray(
        x_pad.reshape(E, NT, 512, HO, P).transpose(0, 1, 4, 3, 2)
    ).reshape(N_CORES, EPC * NT, P, HO, 512)

    # W1/W2 interleaved: [E][it][p][2][ho][128i] with h = ho*128 + p
    def tile_w12(w):
        wb = np.asarray(w, dtype=BF16)
        return np.ascontiguousarray(
            wb.reshape(E, HO, P, IT, P).transpose(0, 3, 2, 1, 4))

    w12_all = np.ascontiguousarray(
        np.stack([tile_w12(gate_w), tile_w12(up_w)], axis=3)
    ).reshape(N_CORES, EPC, IT, P, 2, HO, P)

    # W3 layout: [E][hc][p][io][1024h] with i = io*128 + p
    w3b = np.asarray(down_w, dtype=BF16)
    w3_all = np.ascontiguousarray(
        w3b.reshape(E, IT, P, HC2, 1024).transpose(0, 3, 2, 1, 4)
    ).reshape(N_CORES, EPC, HC2, P, IT, 1024)

    in_maps = [
        {"xt": xt_all[c], "w12": w12_all[c], "w3": w3_all[c]}
        for c in range(N_CORES)
    ]
    return in_maps, offsets, gs


def _run(hidden_states, gate_w, up_w, down_w, group_sizes, trace=False):
    from concourse.bass_utils import run_bass_kernel_spmd

    gs = np.asarray(group_sizes, dtype=np.int64)
    max_g = int(gs.max()) if gs.size else 512
    C = max(512, -(-max_g // 512) * 512)  # round up to multiple of 512

    key = ("v2", C)
    if key not in _PROGRAM_CACHE:
        _PROGRAM_CACHE[key] = _build_program(C)
    nc = _PROGRAM_CACHE[key]

    in_maps, offsets, gs = _prep_inputs(
        hidden_states, gate_w, up_w, down_w, group_sizes, C)

    res = run_bass_kernel_spmd(nc, in_maps, core_ids=list(range(N_CORES)),
                               trace=trace)

    T = hidden_states.shape[0]
    out_full = np.empty((T, H), dtype=np.float32)
    for c in range(N_CORES):
        core_out = res.results[c]["out"]  # [EPC*C, H] bf16
        for el in range(EPC):
            e = c * EPC + el
            out_full[offsets[e]:offsets[e + 1]] = \
                core_out[el * C: el * C + gs[e]].astype(np.float32)
    return out_full, res.exec_time_ns


def kernel(hidden_states, gate_w, up_w, down_w, group_sizes):
    out, _ = _run(hidden_states, gate_w, up_w, down_w, group_sizes)
    return out
